# revision 27
# baseline (speedup 1.0000x reference)
"""Multi-head attention (B=4, N=2048, C=768, H=12) on 8 Trainium2 NeuronCores.

Sharding: core c = (batch b = c//2, head-group g = c%2 of 6 heads).
Each core: qkv projection for its (b, g), attention for 6 heads, partial
output projection against w_proj[:, g-cols]. Host sums the two partial
projections per batch, adds bias, transposes. No collectives.

Device layouts (everything pre-transposed on host; no on-device transposes):
  xT   [768, 2048]   x[b].T              (c on partitions)  bf16
  wqk  [768, 768]    [w_q_g; w_k_g].T    (c on partitions, o free)  bf16
  wv   [768, 384]    w_v_g.T  bf16
  wp   [384, 768]    w_proj[:, gcols].T  (f on partitions, o free)  bf16
  out  [768, 2048]   partial (w_proj_g @ attn_out).T  fp32

Attention per head h (d=64), transposed-score ("sT") formulation:
  sT[k, q] = (k_h chunk).T-matmul: lhsT = k_h [64, 128], rhs = q_h [64, 1024]
  eT = exp(sT * 1/8)  (ScalarE, psum -> sbuf)
  PV transposed: per q-chunk qc of 128, lhsT = eT[:, qc] [128k, 128q],
      rhs = v_h [128k, 64] -> acc[q, d] psum, accumulated over 16 k-chunks;
      denominator via rhs = ones column [128k, 1] -> dn[q] psum.
      (start=True zeroes a whole 2KB psum bank -> only the first matmul
      touching each bank starts.)
  aT[q, d] = acc * recip(dn[q])   (denominator broadcast along free, DVE)
  out_h[d, q] via DMA XBAR transpose of aT [128, 128] tiles (both heads of a
      pair side by side so the transposed tile covers 128 partitions).

Scheduling: the scores/exp stream runs PRE chunks ahead of PV globally;
ScalarE (exp, ~200us) is the bottleneck engine, so deferred qkv/vt/proj
work items are interleaved one per chunk iteration to keep it fed.  The
deferred qk half-copies must be emitted at least one unit before the score
pump (PRE ahead) reads them -- the window map below encodes those deadlines.
"""

import sys

for _p in ("/opt/trn_rl_repo", "/root/.axon_site/_ro/trn_rl_repo"):
    if _p not in sys.path:
        sys.path.insert(0, _p)

import numpy as np
import ml_dtypes

import concourse.bass as bass
import concourse.bacc as bacc
import concourse.mybir as mybir
import concourse.tile as tile
from concourse.bass_utils import run_bass_kernel_spmd

B, N, C = 4, 2048, 768
H, D = 12, 64
HG = 6          # heads per core
P = 128
NCORES = 8
CK = C // P     # 6 contraction chunks for qkv
NT = N // P     # 16 token chunks
QG = 2          # q-groups of 1024
QW = N // QG    # 1024
QC = QW // P    # 8 q-chunks of 128 per q-group
SCALE = D ** -0.5
KB = HG * D // P  # 3: first k-block index offset

# precision plan: inputs bf16 (qkv matmuls accumulate fp32); all matmul
# operands bf16 (validated ~5e-3 rel_l2 end to end)
R_DT = mybir.dt.float32r
B_DT = mybir.dt.bfloat16
NP_BF = ml_dtypes.bfloat16

# bit-trick exp on DVE/GpSimd: exp(s/8) = 2^y, y = s*c; z = round(y) via the
# 1.5*2^23 magic add; 2^f ~ c2*(f - B)^2 + d on [-.5, .5] (max rel 2.2e-3);
# 2^z bits = (bits(M + z) << 23) + (127 << 23)  (low bits of M*2^23 vanish
# mod 2^32).  Offloads ~13% of the exp stream off the bottleneck ScalarE.
DVX_C = float(np.log2(np.e) / 8.0)
DVX_M = 12582912.0
DVX_B = -1.473168703
DVX_SQC2 = float(np.sqrt(0.239449394))
DVX_D = 0.480870388
# t_i32 - 0x4B400000 + 127 = z + 127; split into two adds because scalar
# immediates travel as fp32 and the combined constant is not representable

# (pair, qg, ch, e) -> "dve"; spread so the DVE queue never backs up
# (GpSimd lacks TensorScalarPtr ucode, so DVE is the only offload target)
# offload e=1 tiles: their score slab has a full chunk period of slack
# before reuse, so the DVE copy does not stall the score pipeline
OFFLOAD = {}
for _p in range(3):
    for _qg in range(2):
        OFFLOAD[(_p, _qg, 5, 0)] = "dve"
        OFFLOAD[(_p, _qg, 11, 0)] = "dve"

_CACHED_NC = None


def build_nc():
    nc = bacc.Bacc("TRN2", target_bir_lowering=False, debug=False, num_devices=NCORES)
    f32 = mybir.dt.float32

    # inputs arrive pre-tiled on host: partition-major [P, chunks, free]
    xT = nc.declare_dram_parameter("xT", [P, CK, N], B_DT, isOutput=False)
    wqk = nc.declare_dram_parameter("wqk", [P, CK, 2 * HG * D], B_DT, isOutput=False)
    wv = nc.declare_dram_parameter("wv", [P, CK, HG * D], B_DT, isOutput=False)
    wp = nc.declare_dram_parameter("wp", [P, KB, C], B_DT, isOutput=False)
    ident = nc.declare_dram_parameter("ident", [P, P], B_DT, isOutput=False)
    out = nc.declare_dram_parameter("out", [C, N], f32, isOutput=True)

    with tile.TileContext(nc) as tc:
        with (
            tc.tile_pool(name="big", bufs=1) as big,
            tc.tile_pool(name="et", bufs=19) as etp,
            tc.tile_pool(name="atp", bufs=2) as atp,
            tc.tile_pool(name="nrm", bufs=2) as nrm,
            tc.tile_pool(name="stg", bufs=4) as stg,
            tc.tile_pool(name="scr", bufs=1) as scr,
            tc.tile_pool(name="psS", bufs=2, space="PSUM") as psS,
            tc.tile_pool(name="psA", bufs=2, space="PSUM") as psA,
            tc.tile_pool(name="psD", bufs=1, space="PSUM") as psD,
            tc.tile_pool(name="psW", bufs=1, space="PSUM") as psW,
        ):
            # ---------------- loads ----------------
            # the first scores need only wqk columns for blocks 0/KB and the
            # xT n-half 0; those priority slices go first on the HWDGE path,
            # everything else trickles in behind them via the gpsimd SWDGE
            # queue (separate descriptor-generation engine)
            xT_sb = big.tile([P, CK, N], B_DT)
            wqk_sb = big.tile([P, CK, 2 * HG * D], B_DT)
            wv_sb = big.tile([P, CK, HG * D], B_DT)
            for kc in range(CK):
                nc.sync.dma_start(wqk_sb[:, kc, 0:P], wqk[:, kc, 0:P])
                nc.sync.dma_start(
                    wqk_sb[:, kc, KB * P : (KB + 1) * P], wqk[:, kc, KB * P : (KB + 1) * P]
                )
                nc.sync.dma_start(xT_sb[:, kc, 0:QW], xT[:, kc, 0:QW])
            for kc in range(CK):
                nc.gpsimd.dma_start(xT_sb[:, kc, QW:N], xT[:, kc, QW:N])
                nc.gpsimd.dma_start(wqk_sb[:, kc, P : KB * P], wqk[:, kc, P : KB * P])
                nc.gpsimd.dma_start(
                    wqk_sb[:, kc, (KB + 1) * P :], wqk[:, kc, (KB + 1) * P :]
                )
            nc.gpsimd.dma_start(wv_sb, wv[:, :, :])
            wp_sb = big.tile([P, KB, C], B_DT)
            nc.gpsimd.dma_start(wp_sb, wp[:, :, :])
            ident_sb = big.tile([P, P], B_DT)
            nc.gpsimd.dma_start(ident_sb, ident[:, :])

            # warm the ACT exp table (and engine) during the load phase so the
            # first real exp doesn't pay the ~2.7us table-load latency
            warm = nrm.tile([1, 32], f32, tag="warm")
            nc.vector.memset(warm, 0.0)
            nc.scalar.activation(warm, warm, mybir.ActivationFunctionType.Exp,
                                 bias=0.0, scale=1.0)

            # qk[o, n]: o = 6 q-head cols then 6 k-head cols -> 6 partition blocks
            qk_sb = big.tile([P, 2 * KB, N], B_DT)
            # vT[n, f] with per-head ones column: [n, 6*65], col h*65+64 == 1.0
            vT_sb = big.tile([P, NT, HG * (D + 1)], B_DT)
            ones_view = vT_sb.rearrange("p n (h s) -> p n h s", s=D + 1)[:, :, :, D : D + 1]
            nc.vector.memset(ones_view, 1.0)
            # attention outputs [f, n], f = (head, d) -> 3 partition blocks
            out_h = big.tile([P, KB, N], B_DT)

            # ---------------- qkv ----------------
            # upfront pair-0 q/k blocks, two kc-outer passes of two
            # accumulators each (reusing the two score-psum slabs, which are
            # free before attention starts)
            # upfront qkv: all four pair-0 q/k block-halves accumulate
            # kc-outer and fully in parallel -- nh=0 on the two score slabs,
            # nh=1 on four 512-wide halves borrowed from the psA/psD/psW
            # banks (all idle until attention starts).  The nh=0 copies gate
            # the first scores: one on DVE, one on the still-idle ScalarE.
            ups0 = [
                psS.tile([P, QW], f32, tag="s", name=f"up_ps{j}_0")
                for j in range(2)
            ]
            ups1 = [
                psA.tile([P, 512], f32, tag="acc", name="up_h0"),
                psA.tile([P, 512], f32, tag="acc", name="up_h1"),
                psD.tile([P, 512], f32, tag="dn", name="up_h2"),
                psW.tile([P, 512], f32, tag="w", name="up_h3"),
            ]
            blks = [0, KB]
            for kc in range(CK):
                for j, ot in enumerate(blks):
                    for i in range(QW // 512):
                        nc.tensor.matmul(
                            ups0[j][:, i * 512 : (i + 1) * 512],
                            lhsT=wqk_sb[:, kc, ot * P : (ot + 1) * P],
                            rhs=xT_sb[:, kc, i * 512 : (i + 1) * 512],
                            start=(kc == 0),
                            stop=(kc == CK - 1),
                        )
                for h in range(4):
                    ot = blks[h // 2]
                    hf = h % 2
                    nc.tensor.matmul(
                        ups1[h],
                        lhsT=wqk_sb[:, kc, ot * P : (ot + 1) * P],
                        rhs=xT_sb[:, kc, QW + hf * 512 : QW + (hf + 1) * 512],
                        start=(kc == 0),
                        stop=(kc == CK - 1),
                    )
            nc.vector.tensor_copy(qk_sb[:, 0, 0:QW], ups0[0])
            nc.scalar.copy(qk_sb[:, KB, 0:QW], ups0[1])
            for h in range(4):
                ot = blks[h // 2]
                hf = h % 2
                eng = nc.scalar.copy if h == 3 else nc.vector.tensor_copy
                eng(qk_sb[:, ot, QW + hf * 512 : QW + (hf + 1) * 512], ups1[h])

            # deferred qk half-groups: [128, 512] psum (1 bank) per half
            def emit_qk_half(ot, nh, half):
                ps = psW.tile([P, 512], f32, tag="w", name=f"qk_ps{ot}_{nh}_{half}")
                base = nh * QW + half * 512
                for kc in range(CK):
                    nc.tensor.matmul(
                        ps,
                        lhsT=wqk_sb[:, kc, ot * P : (ot + 1) * P],
                        rhs=xT_sb[:, kc, base : base + 512],
                        start=(kc == 0),
                        stop=(kc == CK - 1),
                    )
                nc.vector.tensor_copy(qk_sb[:, ot, base : base + 512], ps)

            def emit_vt_group(nt):
                ps = psW.tile([P, HG * D], f32, tag="w", name=f"vt_ps{nt}")
                for kc in range(CK):
                    nc.tensor.matmul(
                        ps,
                        lhsT=xT_sb[:, kc, nt * P : (nt + 1) * P],
                        rhs=wv_sb[:, kc, :],
                        start=(kc == 0),
                        stop=(kc == CK - 1),
                    )
                nc.vector.tensor_copy(
                    vT_sb.rearrange("p n (h s) -> p n h s", s=D + 1)[:, nt, :, 0:D],
                    ps.rearrange("p (h s) -> p h s", s=D),
                )

            # proj halves on the single psW bank (used inside the (2,1)
            # window where items are spaced a whole chunk apart)
            def emit_proj_half(ot, nh, half):
                ps = psW.tile([P, 512], f32, tag="w", name=f"pj_ps{ot}_{nh}_{half}")
                base = nh * QW + half * 512
                for fc in range(KB):
                    nc.tensor.matmul(
                        ps,
                        lhsT=wp_sb[:, fc, ot * P : (ot + 1) * P],
                        rhs=out_h[:, fc, base : base + 512],
                        start=(fc == 0),
                        stop=(fc == KB - 1),
                    )
                so = stg.tile([P, 512], f32, tag="so", name=f"so{ot}_{nh}_{half}")
                nc.vector.tensor_copy(so, ps)
                nc.sync.dma_start(out[ot * P : (ot + 1) * P, base : base + 512], so)

            # full proj groups on the (by then free) score slabs -- tail only
            def emit_proj_group(ot, nh):
                ps = psS.tile([P, QW], f32, tag="s", name=f"pjg_ps{ot}_{nh}")
                base = nh * QW
                for fc in range(KB):
                    for i in range(QW // 512):
                        nc.tensor.matmul(
                            ps[:, i * 512 : (i + 1) * 512],
                            lhsT=wp_sb[:, fc, ot * P : (ot + 1) * P],
                            rhs=out_h[:, fc, base + i * 512 : base + (i + 1) * 512],
                            start=(fc == 0),
                            stop=(fc == KB - 1),
                        )
                so = stg.tile([P, QW], f32, tag="sog", name=f"sog{ot}_{nh}")
                if ot % 2 == 0:
                    nc.vector.tensor_copy(so, ps)
                else:
                    nc.scalar.copy(so, ps)
                nc.gpsimd.dma_start(out[ot * P : (ot + 1) * P, base : base + QW], so)

            # deferred work items, interleaved one per chunk iteration (run
            # BEFORE that iteration's pump).  Deadlines: the pump runs PRE=8
            # chunks ahead, so pair p's k-block nh1 halves (read by scores
            # (p,0,ch>=8), pumped from iteration ~0 of unit (p,0)) must be
            # emitted in the PREVIOUS unit's window.
            q1, k1 = 1, KB + 1
            q2, k2 = 2, KB + 2
            windows = {
                (0, 0): [(lambda nt=nt: emit_vt_group(nt)) for nt in range(4, NT)],
                (0, 1): [
                    (lambda blk=blk, hf=hf: emit_qk_half(blk, 0, hf))
                    for blk in (q1, k1) for hf in range(2)
                ] + [
                    (lambda hf=hf: emit_qk_half(k1, 1, hf)) for hf in range(2)
                ],
                (1, 0): [
                    (lambda hf=hf: emit_qk_half(q1, 1, hf)) for hf in range(2)
                ] + [
                    (lambda blk=blk, hf=hf: emit_qk_half(blk, 0, hf))
                    for blk in (q2, k2) for hf in range(2)
                ],
                (1, 1): [
                    (lambda hf=hf: emit_qk_half(k2, 1, hf)) for hf in range(2)
                ],
                (2, 0): [
                    (lambda hf=hf: emit_qk_half(q2, 1, hf)) for hf in range(2)
                ],
                # proj for the first q-half runs inside pair 2's last q-group,
                # where out_h[:, :, 0:QW] is already complete
                (2, 1): [
                    (lambda ot=ot, hf=hf: emit_proj_half(ot, 0, hf))
                    for ot in range(C // P) for hf in range(2)
                ],
            }

            # ---------------- attention ----------------
            PRE = 8
            segs = [(p_, qg) for p_ in range(HG // 2) for qg in range(QG)]
            score_queue = [(p_, qg, ch) for (p_, qg) in segs for ch in range(NT)]
            et_tiles = {}
            qpos = [0]

            i32 = mybir.dt.int32
            A = mybir.AluOpType

            dve_backlog = []

            def emit_exp_trick(eT, ps, kind, tagp):
                """exp(ps * SCALE) -> eT (bf16) on DVE instead of the
                bottleneck ScalarE.  The psum->sbuf copy runs immediately
                (frees the score slab); the 7 elementwise ops are deferred to
                the chunk loop (2 per iteration) so they interleave with the
                time-critical DVE copies/normalizes instead of forming one
                long in-order burst."""
                eng = nc.vector
                # slab plan (bufs=1 per tag; a tile never lands on a slab
                # its own producer reads, and z stays live until op6):
                # tag0: s, w; tag1: t, f, bits; tag2: z; tag3: sq
                # the psum->sbuf copy runs immediately (frees the score
                # slab fast); remaining ops go through the backlog.  Offload
                # spacing (>= 5 chunks) keeps scratch-slab allocations in
                # FIFO order despite the early x0 alloc.
                st = {}
                st["x0"] = scr.tile([P, QW], f32, tag=tagp + "0", name="xs0")
                nc.vector.tensor_copy(st["x0"], ps)                 # s

                def op1():
                    st["x1"] = scr.tile([P, QW], f32, tag=tagp + "1", name="xs1")
                    eng.tensor_scalar(st["x1"], st["x0"], DVX_C, DVX_M, A.mult, A.add)  # t
                def op2():
                    st["x2"] = scr.tile([P, QW], f32, tag=tagp + "2", name="xs2")
                    eng.tensor_scalar(st["x2"], st["x1"], DVX_M, None, A.subtract)  # z
                def op3():
                    st["f"] = scr.tile([P, QW], f32, tag=tagp + "1", name="xsf")
                    eng.scalar_tensor_tensor(st["f"], st["x0"], DVX_C, st["x2"], A.mult, A.subtract)
                def op4():
                    st["w"] = scr.tile([P, QW], f32, tag=tagp + "0", name="xsw")
                    eng.tensor_scalar(st["w"], st["f"], -DVX_B, DVX_SQC2, A.add, A.mult)
                def op5():
                    st["sq"] = scr.tile([P, QW], f32, tag=tagp + "3", name="xsq")
                    eng.scalar_tensor_tensor(st["sq"], st["w"], 1.0, st["w"], A.mult, A.mult)
                def op6():
                    # (z+127)*2^23 exactly in fp32; int32 write conversion
                    # yields the bit pattern of 2^z
                    st["bits"] = scr.tile([P, QW], i32, tag=tagp + "1", name="xsb")
                    eng.tensor_scalar(st["bits"], st["x2"], 8388608.0,
                                      127.0 * 8388608.0, A.mult, A.add)
                def op7():
                    eng.scalar_tensor_tensor(eT, st["sq"], DVX_D,
                                             st["bits"].bitcast(f32), A.add, A.mult)

                dve_backlog.extend([op1, op2, op3, op4, op5, op6, op7])

            def drain_dve(k):
                for _ in range(k):
                    if dve_backlog:
                        dve_backlog.pop(0)()

            def emit_scores(sp, sqg, ch):
                qb = sp
                kb = KB + sp
                for e in range(2):
                    base = e * D
                    ps = psS.tile([P, QW], f32, tag="s", name=f"sps{sp}_{sqg}_{ch}_{e}")
                    for i in range(QW // 512):
                        nc.tensor.matmul(
                            ps[:, i * 512 : (i + 1) * 512],
                            lhsT=qk_sb[base : base + D, kb, ch * P : (ch + 1) * P],
                            rhs=qk_sb[base : base + D, qb, sqg * QW + i * 512 : sqg * QW + (i + 1) * 512],
                            start=True,
                            stop=True,
                        )
                    eT = etp.tile([P, QW], B_DT, tag="et", name=f"et{sp}_{sqg}_{ch}_{e}")
                    kind = OFFLOAD.get((sp, sqg, ch, e))
                    if kind is None:
                        nc.scalar.activation(
                            eT, ps, mybir.ActivationFunctionType.Exp,
                            bias=0.0, scale=float(SCALE),
                        )
                    else:
                        emit_exp_trick(eT, ps, kind, "d" if kind == "dve" else "p")
                    et_tiles[(sp, sqg, ch, e)] = eT

            def pump_scores(n):
                for _ in range(n):
                    if qpos[0] < len(score_queue):
                        emit_scores(*score_queue[qpos[0]])
                        qpos[0] += 1

            # pipeline fill: interleave the first vT groups with score
            # chunks so ScalarE starts chewing immediately
            pump_scores(3)
            for nt in range(4):
                emit_vt_group(nt)
                pump_scores(1)
            pump_scores(PRE - 7)

            for p_, qg in segs:
                work = list(windows.get((p_, qg), []))
                wi = [0]

                def run_work(k=1):
                    for _ in range(k):
                        if wi[0] < len(work):
                            work[wi[0]]()
                            wi[0] += 1

                accs = [
                    psA.tile([P, QC * D], f32, tag="acc", name=f"acc{p_}_{qg}_{e}")
                    for e in range(2)
                ]
                dn = psD.tile([P, 2 * QC], f32, tag="dn", name=f"dn{p_}_{qg}")
                for ch in range(NT):
                    pump_scores(1)
                    run_work(1)
                    drain_dve(2)
                    for e in range(2):
                        h = 2 * p_ + e
                        eT = et_tiles.pop((p_, qg, ch, e))
                        vcol = vT_sb[:, ch, h * (D + 1) : h * (D + 1) + D]
                        ocol = vT_sb[:, ch, h * (D + 1) + D : h * (D + 1) + D + 1]
                        for qc in range(QC):
                            lhs = eT[:, qc * P : (qc + 1) * P]
                            nc.tensor.matmul(
                                accs[e][:, qc * D : (qc + 1) * D],
                                lhsT=lhs,
                                rhs=vcol,
                                start=(ch == 0 and qc == 0),
                                stop=(ch == NT - 1),
                            )
                            nc.tensor.matmul(
                                dn[:, e * QC + qc : e * QC + qc + 1],
                                lhsT=lhs,
                                rhs=ocol,
                                start=(ch == 0 and qc == 0 and e == 0),
                                stop=(ch == NT - 1),
                            )
                # finish any window leftovers
                run_work(len(work))

                # normalize: aT[q, f] = acc[q, d] * recip(dn[q]) (denominator
                # broadcast along free; DVE can read only one psum operand, so
                # the reciprocal lands in SBUF first); both heads share one aT
                # tile so the transpose covers 128 partitions
                dnr = nrm.tile([P, 2 * QC], f32, tag="dnr", name=f"dnr{p_}_{qg}")
                nc.vector.reciprocal(dnr, dn)
                aT = atp.tile([P, QC, P], B_DT, tag="at", name=f"aT{p_}_{qg}")
                for e in range(2):
                    dslice = dnr[:, e * QC : (e + 1) * QC]
                    bcast = dslice.unsqueeze(-1).broadcast_to((P, QC, D))
                    nc.vector.tensor_tensor(
                        aT[:, :, e * D : (e + 1) * D],
                        accs[e].rearrange("p (a b) -> p a b", b=D),
                        bcast,
                        mybir.AluOpType.mult,
                    )
                if (p_, qg) != segs[-1]:
                    # DMA XBAR transpose: [128 q, 128 f] -> out_h [128 f, 128 q]
                    for qc in range(QC):
                        nc.sync.dma_start(
                            out_h[:, p_, qg * QW + qc * P : qg * QW + (qc + 1) * P],
                            aT[:, qc, :],
                            transpose=True,
                        )
                else:
                    # last unit: the HWDGE transposes would serialize on the
                    # kernel tail; PE is idle by now, so transpose via the
                    # identity-matmul path into the freed psW/psD banks
                    for e in range(2):
                        tp = [
                            psW.tile([64, 512], B_DT, tag="w", name=f"tp{e}0"),
                            psD.tile([64, 512], B_DT, tag="dn", name=f"tp{e}1"),
                        ]
                        for qc in range(QC):
                            nc.tensor.matmul(
                                tp[qc // 4][0:64, (qc % 4) * P : (qc % 4 + 1) * P],
                                lhsT=aT[:, qc, e * D : (e + 1) * D],
                                rhs=ident_sb,
                                is_transpose=True,
                                start=(qc % 4 == 0),
                                stop=(qc % 4 == 3),
                            )
                        for half in range(2):
                            eng = nc.vector.tensor_copy if half == 0 else nc.scalar.copy
                            eng(
                                out_h[e * D : (e + 1) * D, p_,
                                      qg * QW + half * 512 : qg * QW + (half + 1) * 512],
                                tp[half],
                            )

            drain_dve(len(dve_backlog))
            # ---------------- proj (nh=1 half; nh=0 ran in the (2,1) window) --
            # full groups rotating over the two freed score slabs
            for ot in range(C // P):
                emit_proj_group(ot, 1)
    nc.compile()
    return nc


def _get_nc():
    global _CACHED_NC
    if _CACHED_NC is None:
        _CACHED_NC = build_nc()
    return _CACHED_NC


def shard_inputs(x, w_qkv, w_proj):
    """Build per-core input maps from full inputs."""
    in_maps = []
    for c in range(NCORES):
        b, g = divmod(c, 2)
        r = slice(HG * D * g, HG * D * (g + 1))
        def ptile(m):
            return np.ascontiguousarray(m.reshape(m.shape[0] // P, P, m.shape[1]).transpose(1, 0, 2))
        xT = ptile(x[b].T.astype(NP_BF))
        wq = w_qkv[r]
        wk = w_qkv[C + HG * D * g : C + HG * D * (g + 1)]
        wv_ = w_qkv[2 * C + HG * D * g : 2 * C + HG * D * (g + 1)]
        wqk = ptile(np.concatenate([wq, wk], axis=0).T.astype(NP_BF))
        wvT = ptile(wv_.T.astype(NP_BF))
        wpT = ptile(w_proj[:, r].T.astype(NP_BF))
        in_maps.append({"xT": xT, "wqk": wqk, "wv": wvT, "wp": wpT,
                        "ident": np.eye(P, dtype=NP_BF)})
    return in_maps


def run(x, w_qkv, w_proj, b_proj, trace=False):
    nc = _get_nc()
    in_maps = shard_inputs(x, w_qkv, w_proj)
    try:
        res = run_bass_kernel_spmd(nc, in_maps, list(range(NCORES)), trace=trace)
    except Exception:
        # one retry for transient runtime/tunnel hiccups
        res = run_bass_kernel_spmd(nc, in_maps, list(range(NCORES)), trace=trace)
    y = np.empty((B, N, C), np.float32)
    for b in range(B):
        part = res.results[2 * b]["out"] + res.results[2 * b + 1]["out"]
        y[b] = part.T + b_proj.astype(np.float32)
    return y, res


def kernel(x, w_qkv, w_proj, b_proj):
    x = np.asarray(x, dtype=np.float32)
    w_qkv = np.asarray(w_qkv, dtype=np.float32)
    w_proj = np.asarray(w_proj, dtype=np.float32)
    b_proj = np.asarray(b_proj, dtype=np.float32)
    y, _ = run(x, w_qkv, w_proj, b_proj, trace=False)
    return y


# revision 28
# speedup vs baseline: 1.0300x; 1.0300x over previous
"""Multi-head attention (B=4, N=2048, C=768, H=12) on 8 Trainium2 NeuronCores.

Sharding: core c = (batch b = c//2, head-group g = c%2 of 6 heads).
Each core: qkv projection for its (b, g), attention for 6 heads, partial
output projection against w_proj[:, g-cols]. Host sums the two partial
projections per batch, adds bias, transposes. No collectives.

Device layouts (everything pre-transposed on host; no on-device transposes):
  xT   [768, 2048]   x[b].T              (c on partitions)  bf16
  wqk  [768, 768]    [w_q_g; w_k_g].T    (c on partitions, o free)  bf16
  wv   [768, 384]    w_v_g.T  bf16
  wp   [384, 768]    w_proj[:, gcols].T  (f on partitions, o free)  bf16
  out  [768, 2048]   partial (w_proj_g @ attn_out).T  fp32

Attention per head h (d=64), transposed-score ("sT") formulation:
  sT[k, q] = (k_h chunk).T-matmul: lhsT = k_h [64, 128], rhs = q_h [64, 1024]
  eT = exp(sT * 1/8)  (ScalarE, psum -> sbuf)
  PV transposed: per q-chunk qc of 128, lhsT = eT[:, qc] [128k, 128q],
      rhs = v_h [128k, 64] -> acc[q, d] psum, accumulated over 16 k-chunks;
      denominator via rhs = ones column [128k, 1] -> dn[q] psum.
      (start=True zeroes a whole 2KB psum bank -> only the first matmul
      touching each bank starts.)
  aT[q, d] = acc * recip(dn[q])   (denominator broadcast along free, DVE)
  out_h[d, q] via DMA XBAR transpose of aT [128, 128] tiles (both heads of a
      pair side by side so the transposed tile covers 128 partitions).

Scheduling: the scores/exp stream runs PRE chunks ahead of PV globally;
ScalarE (exp, ~200us) is the bottleneck engine, so deferred qkv/vt/proj
work items are interleaved one per chunk iteration to keep it fed.  The
deferred qk half-copies must be emitted at least one unit before the score
pump (PRE ahead) reads them -- the window map below encodes those deadlines.
"""

import sys

for _p in ("/opt/trn_rl_repo", "/root/.axon_site/_ro/trn_rl_repo"):
    if _p not in sys.path:
        sys.path.insert(0, _p)

import numpy as np
import ml_dtypes

import concourse.bass as bass
import concourse.bacc as bacc
import concourse.mybir as mybir
import concourse.tile as tile
from concourse.bass_utils import run_bass_kernel_spmd

B, N, C = 4, 2048, 768
H, D = 12, 64
HG = 6          # heads per core
P = 128
NCORES = 8
CK = C // P     # 6 contraction chunks for qkv
NT = N // P     # 16 token chunks
QG = 2          # q-groups of 1024
QW = N // QG    # 1024
QC = QW // P    # 8 q-chunks of 128 per q-group
SCALE = D ** -0.5
KB = HG * D // P  # 3: first k-block index offset

# precision plan: inputs bf16 (qkv matmuls accumulate fp32); all matmul
# operands bf16 (validated ~5e-3 rel_l2 end to end)
R_DT = mybir.dt.float32r
B_DT = mybir.dt.bfloat16
NP_BF = ml_dtypes.bfloat16

# bit-trick exp on DVE/GpSimd: exp(s/8) = 2^y, y = s*c; z = round(y) via the
# 1.5*2^23 magic add; 2^f ~ c2*(f - B)^2 + d on [-.5, .5] (max rel 2.2e-3);
# 2^z bits = (bits(M + z) << 23) + (127 << 23)  (low bits of M*2^23 vanish
# mod 2^32).  Offloads ~13% of the exp stream off the bottleneck ScalarE.
DVX_C = float(np.log2(np.e) / 8.0)
DVX_M = 12582912.0
DVX_B = -1.473168703
DVX_SQC2 = float(np.sqrt(0.239449394))
DVX_D = 0.480870388
# t_i32 - 0x4B400000 + 127 = z + 127; split into two adds because scalar
# immediates travel as fp32 and the combined constant is not representable

# (pair, qg, ch, e) -> "dve"; spread so the DVE queue never backs up
# (GpSimd lacks TensorScalarPtr ucode, so DVE is the only offload target)
# offload e=1 tiles: their score slab has a full chunk period of slack
# before reuse, so the DVE copy does not stall the score pipeline
OFFLOAD = {}
for _p in range(3):
    for _qg in range(2):
        OFFLOAD[(_p, _qg, 5, 0)] = "dve"
        OFFLOAD[(_p, _qg, 11, 0)] = "dve"

_CACHED_NC = None


def build_nc():
    nc = bacc.Bacc("TRN2", target_bir_lowering=False, debug=False, num_devices=NCORES)
    f32 = mybir.dt.float32

    # inputs arrive pre-tiled on host: partition-major [P, chunks, free]
    xT = nc.declare_dram_parameter("xT", [P, CK, N], B_DT, isOutput=False)
    wqk = nc.declare_dram_parameter("wqk", [P, CK, 2 * HG * D], B_DT, isOutput=False)
    wv = nc.declare_dram_parameter("wv", [P, CK, HG * D], B_DT, isOutput=False)
    wp = nc.declare_dram_parameter("wp", [P, KB, C], B_DT, isOutput=False)
    ident = nc.declare_dram_parameter("ident", [P, P], B_DT, isOutput=False)
    out = nc.declare_dram_parameter("out", [C, N], f32, isOutput=True)

    with tile.TileContext(nc) as tc:
        with (
            tc.tile_pool(name="big", bufs=1) as big,
            tc.tile_pool(name="et", bufs=19) as etp,
            tc.tile_pool(name="atp", bufs=2) as atp,
            tc.tile_pool(name="nrm", bufs=2) as nrm,
            tc.tile_pool(name="stg", bufs=4) as stg,
            tc.tile_pool(name="scr", bufs=1) as scr,
            tc.tile_pool(name="psS", bufs=2, space="PSUM") as psS,
            tc.tile_pool(name="psA", bufs=2, space="PSUM") as psA,
            tc.tile_pool(name="psD", bufs=1, space="PSUM") as psD,
            tc.tile_pool(name="psW", bufs=1, space="PSUM") as psW,
        ):
            # ---------------- loads ----------------
            # the first scores need only wqk columns for blocks 0/KB and the
            # xT n-half 0; those priority slices go first on the HWDGE path,
            # everything else trickles in behind them via the gpsimd SWDGE
            # queue (separate descriptor-generation engine)
            xT_sb = big.tile([P, CK, N], B_DT)
            wqk_sb = big.tile([P, CK, 2 * HG * D], B_DT)
            wv_sb = big.tile([P, CK, HG * D], B_DT)
            for kc in range(CK):
                nc.sync.dma_start(wqk_sb[:, kc, 0 : 2 * P], wqk[:, kc, 0 : 2 * P])
                nc.sync.dma_start(xT_sb[:, kc, 0:QW], xT[:, kc, 0:QW])
            for kc in range(CK):
                nc.gpsimd.dma_start(xT_sb[:, kc, QW:N], xT[:, kc, QW:N])
            nc.gpsimd.dma_start(wv_sb, wv[:, :, :])
            for kc in range(CK):
                nc.gpsimd.dma_start(wqk_sb[:, kc, 2 * P :], wqk[:, kc, 2 * P :])
            wp_sb = big.tile([P, KB, C], B_DT)
            nc.gpsimd.dma_start(wp_sb, wp[:, :, :])
            ident_sb = big.tile([P, P], B_DT)
            nc.gpsimd.dma_start(ident_sb, ident[:, :])

            # warm the ACT exp table (and engine) during the load phase so the
            # first real exp doesn't pay the ~2.7us table-load latency
            warm = nrm.tile([1, 32], f32, tag="warm")
            nc.vector.memset(warm, 0.0)
            nc.scalar.activation(warm, warm, mybir.ActivationFunctionType.Exp,
                                 bias=0.0, scale=1.0)

            # qk[o, n]: o = 6 q-head cols then 6 k-head cols -> 6 partition blocks
            qk_sb = big.tile([P, 2 * KB, N], B_DT)
            # vT[n, f] with per-head ones column: [n, 6*65], col h*65+64 == 1.0
            vT_sb = big.tile([P, NT, HG * (D + 1)], B_DT)
            ones_view = vT_sb.rearrange("p n (h s) -> p n h s", s=D + 1)[:, :, :, D : D + 1]
            nc.vector.memset(ones_view, 1.0)
            # attention outputs [f, n], f = (head, d) -> 3 partition blocks
            out_h = big.tile([P, KB, N], B_DT)

            # ---------------- qkv ----------------
            # upfront pair-0 q/k blocks, two kc-outer passes of two
            # accumulators each (reusing the two score-psum slabs, which are
            # free before attention starts)
            # upfront qkv: all four pair-0 q/k block-halves accumulate
            # kc-outer and fully in parallel -- nh=0 on the two score slabs,
            # nh=1 on four 512-wide halves borrowed from the psA/psD/psW
            # banks (all idle until attention starts).  The nh=0 copies gate
            # the first scores: one on DVE, one on the still-idle ScalarE.
            ups0 = [
                psS.tile([P, QW], f32, tag="s", name=f"up_ps{j}_0")
                for j in range(2)
            ]
            ups1 = [
                psA.tile([P, 512], f32, tag="acc", name="up_h0"),
                psA.tile([P, 512], f32, tag="acc", name="up_h1"),
                psD.tile([P, 512], f32, tag="dn", name="up_h2"),
                psW.tile([P, 512], f32, tag="w", name="up_h3"),
            ]
            blks = [0, 1]
            for kc in range(CK):
                for j, ot in enumerate(blks):
                    for i in range(QW // 512):
                        nc.tensor.matmul(
                            ups0[j][:, i * 512 : (i + 1) * 512],
                            lhsT=wqk_sb[:, kc, ot * P : (ot + 1) * P],
                            rhs=xT_sb[:, kc, i * 512 : (i + 1) * 512],
                            start=(kc == 0),
                            stop=(kc == CK - 1),
                        )
                for h in range(4):
                    ot = blks[h // 2]
                    hf = h % 2
                    nc.tensor.matmul(
                        ups1[h],
                        lhsT=wqk_sb[:, kc, ot * P : (ot + 1) * P],
                        rhs=xT_sb[:, kc, QW + hf * 512 : QW + (hf + 1) * 512],
                        start=(kc == 0),
                        stop=(kc == CK - 1),
                    )
            nc.vector.tensor_copy(qk_sb[:, 0, 0:QW], ups0[0])
            nc.scalar.copy(qk_sb[:, 1, 0:QW], ups0[1])
            for h in range(4):
                ot = blks[h // 2]
                hf = h % 2
                eng = nc.scalar.copy if h == 3 else nc.vector.tensor_copy
                eng(qk_sb[:, ot, QW + hf * 512 : QW + (hf + 1) * 512], ups1[h])

            # deferred qk half-groups: [128, 512] psum (1 bank) per half
            def emit_qk_half(ot, nh, half):
                ps = psW.tile([P, 512], f32, tag="w", name=f"qk_ps{ot}_{nh}_{half}")
                base = nh * QW + half * 512
                for kc in range(CK):
                    nc.tensor.matmul(
                        ps,
                        lhsT=wqk_sb[:, kc, ot * P : (ot + 1) * P],
                        rhs=xT_sb[:, kc, base : base + 512],
                        start=(kc == 0),
                        stop=(kc == CK - 1),
                    )
                nc.vector.tensor_copy(qk_sb[:, ot, base : base + 512], ps)

            def emit_vt_group(nt):
                ps = psW.tile([P, HG * D], f32, tag="w", name=f"vt_ps{nt}")
                for kc in range(CK):
                    nc.tensor.matmul(
                        ps,
                        lhsT=xT_sb[:, kc, nt * P : (nt + 1) * P],
                        rhs=wv_sb[:, kc, :],
                        start=(kc == 0),
                        stop=(kc == CK - 1),
                    )
                nc.vector.tensor_copy(
                    vT_sb.rearrange("p n (h s) -> p n h s", s=D + 1)[:, nt, :, 0:D],
                    ps.rearrange("p (h s) -> p h s", s=D),
                )

            # proj halves on the single psW bank (used inside the (2,1)
            # window where items are spaced a whole chunk apart)
            def emit_proj_half(ot, nh, half):
                ps = psW.tile([P, 512], f32, tag="w", name=f"pj_ps{ot}_{nh}_{half}")
                base = nh * QW + half * 512
                for fc in range(KB):
                    nc.tensor.matmul(
                        ps,
                        lhsT=wp_sb[:, fc, ot * P : (ot + 1) * P],
                        rhs=out_h[:, fc, base : base + 512],
                        start=(fc == 0),
                        stop=(fc == KB - 1),
                    )
                so = stg.tile([P, 512], f32, tag="so", name=f"so{ot}_{nh}_{half}")
                nc.vector.tensor_copy(so, ps)
                nc.sync.dma_start(out[ot * P : (ot + 1) * P, base : base + 512], so)

            # full proj groups on the (by then free) score slabs -- tail only
            def emit_proj_group(ot, nh):
                ps = psS.tile([P, QW], f32, tag="s", name=f"pjg_ps{ot}_{nh}")
                base = nh * QW
                for fc in range(KB):
                    for i in range(QW // 512):
                        nc.tensor.matmul(
                            ps[:, i * 512 : (i + 1) * 512],
                            lhsT=wp_sb[:, fc, ot * P : (ot + 1) * P],
                            rhs=out_h[:, fc, base + i * 512 : base + (i + 1) * 512],
                            start=(fc == 0),
                            stop=(fc == KB - 1),
                        )
                so = stg.tile([P, QW], f32, tag="sog", name=f"sog{ot}_{nh}")
                if ot % 2 == 0:
                    nc.vector.tensor_copy(so, ps)
                else:
                    nc.scalar.copy(so, ps)
                nc.gpsimd.dma_start(out[ot * P : (ot + 1) * P, base : base + QW], so)

            # deferred work items, interleaved one per chunk iteration (run
            # BEFORE that iteration's pump).  Deadlines: the pump runs PRE=8
            # chunks ahead, so pair p's k-block nh1 halves (read by scores
            # (p,0,ch>=8), pumped from iteration ~0 of unit (p,0)) must be
            # emitted in the PREVIOUS unit's window.
            q1, k1 = 2, 3
            q2, k2 = 4, 5
            windows = {
                (0, 0): [(lambda nt=nt: emit_vt_group(nt)) for nt in range(4, NT)],
                (0, 1): [
                    (lambda blk=blk, hf=hf: emit_qk_half(blk, 0, hf))
                    for blk in (q1, k1) for hf in range(2)
                ] + [
                    (lambda hf=hf: emit_qk_half(k1, 1, hf)) for hf in range(2)
                ],
                (1, 0): [
                    (lambda hf=hf: emit_qk_half(q1, 1, hf)) for hf in range(2)
                ] + [
                    (lambda blk=blk, hf=hf: emit_qk_half(blk, 0, hf))
                    for blk in (q2, k2) for hf in range(2)
                ],
                (1, 1): [
                    (lambda hf=hf: emit_qk_half(k2, 1, hf)) for hf in range(2)
                ],
                (2, 0): [
                    (lambda hf=hf: emit_qk_half(q2, 1, hf)) for hf in range(2)
                ],
                # proj for the first q-half runs inside pair 2's last q-group,
                # where out_h[:, :, 0:QW] is already complete
                (2, 1): [
                    (lambda ot=ot, hf=hf: emit_proj_half(ot, 0, hf))
                    for ot in range(C // P) for hf in range(2)
                ],
            }

            # ---------------- attention ----------------
            PRE = 8
            segs = [(p_, qg) for p_ in range(HG // 2) for qg in range(QG)]
            score_queue = [(p_, qg, ch) for (p_, qg) in segs for ch in range(NT)]
            et_tiles = {}
            qpos = [0]

            i32 = mybir.dt.int32
            A = mybir.AluOpType

            dve_backlog = []

            def emit_exp_trick(eT, ps, kind, tagp):
                """exp(ps * SCALE) -> eT (bf16) on DVE instead of the
                bottleneck ScalarE.  The psum->sbuf copy runs immediately
                (frees the score slab); the 7 elementwise ops are deferred to
                the chunk loop (2 per iteration) so they interleave with the
                time-critical DVE copies/normalizes instead of forming one
                long in-order burst."""
                eng = nc.vector
                # slab plan (bufs=1 per tag; a tile never lands on a slab
                # its own producer reads, and z stays live until op6):
                # tag0: s, w; tag1: t, f, bits; tag2: z; tag3: sq
                # the psum->sbuf copy runs immediately (frees the score
                # slab fast); remaining ops go through the backlog.  Offload
                # spacing (>= 5 chunks) keeps scratch-slab allocations in
                # FIFO order despite the early x0 alloc.
                st = {}
                st["x0"] = scr.tile([P, QW], f32, tag=tagp + "0", name="xs0")
                nc.vector.tensor_copy(st["x0"], ps)                 # s

                def op1():
                    st["x1"] = scr.tile([P, QW], f32, tag=tagp + "1", name="xs1")
                    eng.tensor_scalar(st["x1"], st["x0"], DVX_C, DVX_M, A.mult, A.add)  # t
                def op2():
                    st["x2"] = scr.tile([P, QW], f32, tag=tagp + "2", name="xs2")
                    eng.tensor_scalar(st["x2"], st["x1"], DVX_M, None, A.subtract)  # z
                def op3():
                    st["f"] = scr.tile([P, QW], f32, tag=tagp + "1", name="xsf")
                    eng.scalar_tensor_tensor(st["f"], st["x0"], DVX_C, st["x2"], A.mult, A.subtract)
                def op4():
                    st["w"] = scr.tile([P, QW], f32, tag=tagp + "0", name="xsw")
                    eng.tensor_scalar(st["w"], st["f"], -DVX_B, DVX_SQC2, A.add, A.mult)
                def op5():
                    st["sq"] = scr.tile([P, QW], f32, tag=tagp + "3", name="xsq")
                    eng.scalar_tensor_tensor(st["sq"], st["w"], 1.0, st["w"], A.mult, A.mult)
                def op6():
                    # (z+127)*2^23 exactly in fp32; int32 write conversion
                    # yields the bit pattern of 2^z
                    st["bits"] = scr.tile([P, QW], i32, tag=tagp + "1", name="xsb")
                    eng.tensor_scalar(st["bits"], st["x2"], 8388608.0,
                                      127.0 * 8388608.0, A.mult, A.add)
                def op7():
                    eng.scalar_tensor_tensor(eT, st["sq"], DVX_D,
                                             st["bits"].bitcast(f32), A.add, A.mult)

                dve_backlog.extend([op1, op2, op3, op4, op5, op6, op7])

            def drain_dve(k):
                for _ in range(k):
                    if dve_backlog:
                        dve_backlog.pop(0)()

            def emit_scores(sp, sqg, ch):
                qb = 2 * sp
                kb = 2 * sp + 1
                for e in range(2):
                    base = e * D
                    ps = psS.tile([P, QW], f32, tag="s", name=f"sps{sp}_{sqg}_{ch}_{e}")
                    for i in range(QW // 512):
                        nc.tensor.matmul(
                            ps[:, i * 512 : (i + 1) * 512],
                            lhsT=qk_sb[base : base + D, kb, ch * P : (ch + 1) * P],
                            rhs=qk_sb[base : base + D, qb, sqg * QW + i * 512 : sqg * QW + (i + 1) * 512],
                            start=True,
                            stop=True,
                        )
                    eT = etp.tile([P, QW], B_DT, tag="et", name=f"et{sp}_{sqg}_{ch}_{e}")
                    kind = OFFLOAD.get((sp, sqg, ch, e))
                    if kind is None:
                        nc.scalar.activation(
                            eT, ps, mybir.ActivationFunctionType.Exp,
                            bias=0.0, scale=float(SCALE),
                        )
                    else:
                        emit_exp_trick(eT, ps, kind, "d" if kind == "dve" else "p")
                    et_tiles[(sp, sqg, ch, e)] = eT

            def pump_scores(n):
                for _ in range(n):
                    if qpos[0] < len(score_queue):
                        emit_scores(*score_queue[qpos[0]])
                        qpos[0] += 1

            # pipeline fill: interleave the first vT groups with score
            # chunks so ScalarE starts chewing immediately
            pump_scores(3)
            for nt in range(4):
                emit_vt_group(nt)
                pump_scores(1)
            pump_scores(PRE - 7)

            for p_, qg in segs:
                work = list(windows.get((p_, qg), []))
                wi = [0]

                def run_work(k=1):
                    for _ in range(k):
                        if wi[0] < len(work):
                            work[wi[0]]()
                            wi[0] += 1

                accs = [
                    psA.tile([P, QC * D], f32, tag="acc", name=f"acc{p_}_{qg}_{e}")
                    for e in range(2)
                ]
                dn = psD.tile([P, 2 * QC], f32, tag="dn", name=f"dn{p_}_{qg}")
                for ch in range(NT):
                    pump_scores(1)
                    run_work(1)
                    drain_dve(2)
                    for e in range(2):
                        h = 2 * p_ + e
                        eT = et_tiles.pop((p_, qg, ch, e))
                        vcol = vT_sb[:, ch, h * (D + 1) : h * (D + 1) + D]
                        ocol = vT_sb[:, ch, h * (D + 1) + D : h * (D + 1) + D + 1]
                        for qc in range(QC):
                            lhs = eT[:, qc * P : (qc + 1) * P]
                            nc.tensor.matmul(
                                accs[e][:, qc * D : (qc + 1) * D],
                                lhsT=lhs,
                                rhs=vcol,
                                start=(ch == 0 and qc == 0),
                                stop=(ch == NT - 1),
                            )
                            nc.tensor.matmul(
                                dn[:, e * QC + qc : e * QC + qc + 1],
                                lhsT=lhs,
                                rhs=ocol,
                                start=(ch == 0 and qc == 0 and e == 0),
                                stop=(ch == NT - 1),
                            )
                # finish any window leftovers
                run_work(len(work))

                # normalize: aT[q, f] = acc[q, d] * recip(dn[q]) (denominator
                # broadcast along free; DVE can read only one psum operand, so
                # the reciprocal lands in SBUF first); both heads share one aT
                # tile so the transpose covers 128 partitions
                dnr = nrm.tile([P, 2 * QC], f32, tag="dnr", name=f"dnr{p_}_{qg}")
                nc.vector.reciprocal(dnr, dn)
                aT = atp.tile([P, QC, P], B_DT, tag="at", name=f"aT{p_}_{qg}")
                for e in range(2):
                    dslice = dnr[:, e * QC : (e + 1) * QC]
                    bcast = dslice.unsqueeze(-1).broadcast_to((P, QC, D))
                    nc.vector.tensor_tensor(
                        aT[:, :, e * D : (e + 1) * D],
                        accs[e].rearrange("p (a b) -> p a b", b=D),
                        bcast,
                        mybir.AluOpType.mult,
                    )
                if (p_, qg) != segs[-1]:
                    # DMA XBAR transpose: [128 q, 128 f] -> out_h [128 f, 128 q]
                    for qc in range(QC):
                        nc.sync.dma_start(
                            out_h[:, p_, qg * QW + qc * P : qg * QW + (qc + 1) * P],
                            aT[:, qc, :],
                            transpose=True,
                        )
                else:
                    # last unit: the HWDGE transposes would serialize on the
                    # kernel tail; PE is idle by now, so transpose via the
                    # identity-matmul path into the freed psW/psD banks
                    for e in range(2):
                        tp = [
                            psW.tile([64, 512], B_DT, tag="w", name=f"tp{e}0"),
                            psD.tile([64, 512], B_DT, tag="dn", name=f"tp{e}1"),
                        ]
                        for qc in range(QC):
                            nc.tensor.matmul(
                                tp[qc // 4][0:64, (qc % 4) * P : (qc % 4 + 1) * P],
                                lhsT=aT[:, qc, e * D : (e + 1) * D],
                                rhs=ident_sb,
                                is_transpose=True,
                                start=(qc % 4 == 0),
                                stop=(qc % 4 == 3),
                            )
                        for half in range(2):
                            eng = nc.vector.tensor_copy if half == 0 else nc.scalar.copy
                            eng(
                                out_h[e * D : (e + 1) * D, p_,
                                      qg * QW + half * 512 : qg * QW + (half + 1) * 512],
                                tp[half],
                            )

            drain_dve(len(dve_backlog))
            # ---------------- proj (nh=1 half; nh=0 ran in the (2,1) window) --
            # full groups rotating over the two freed score slabs
            for ot in range(C // P):
                emit_proj_group(ot, 1)
    nc.compile()
    return nc


def _get_nc():
    global _CACHED_NC
    if _CACHED_NC is None:
        _CACHED_NC = build_nc()
    return _CACHED_NC


def shard_inputs(x, w_qkv, w_proj):
    """Build per-core input maps from full inputs."""
    in_maps = []
    for c in range(NCORES):
        b, g = divmod(c, 2)
        r = slice(HG * D * g, HG * D * (g + 1))
        def ptile(m):
            return np.ascontiguousarray(m.reshape(m.shape[0] // P, P, m.shape[1]).transpose(1, 0, 2))
        xT = ptile(x[b].T.astype(NP_BF))
        wq = w_qkv[r]
        wk = w_qkv[C + HG * D * g : C + HG * D * (g + 1)]
        wv_ = w_qkv[2 * C + HG * D * g : 2 * C + HG * D * (g + 1)]
        wqi = np.concatenate(
            [m for p in range(3) for m in (wq[p * P : (p + 1) * P], wk[p * P : (p + 1) * P])],
            axis=0,
        )
        wqk = ptile(wqi.T.astype(NP_BF))
        wvT = ptile(wv_.T.astype(NP_BF))
        wpT = ptile(w_proj[:, r].T.astype(NP_BF))
        in_maps.append({"xT": xT, "wqk": wqk, "wv": wvT, "wp": wpT,
                        "ident": np.eye(P, dtype=NP_BF)})
    return in_maps


def run(x, w_qkv, w_proj, b_proj, trace=False):
    nc = _get_nc()
    in_maps = shard_inputs(x, w_qkv, w_proj)
    try:
        res = run_bass_kernel_spmd(nc, in_maps, list(range(NCORES)), trace=trace)
    except Exception:
        # one retry for transient runtime/tunnel hiccups
        res = run_bass_kernel_spmd(nc, in_maps, list(range(NCORES)), trace=trace)
    y = np.empty((B, N, C), np.float32)
    for b in range(B):
        part = res.results[2 * b]["out"] + res.results[2 * b + 1]["out"]
        y[b] = part.T + b_proj.astype(np.float32)
    return y, res


def kernel(x, w_qkv, w_proj, b_proj):
    x = np.asarray(x, dtype=np.float32)
    w_qkv = np.asarray(w_qkv, dtype=np.float32)
    w_proj = np.asarray(w_proj, dtype=np.float32)
    b_proj = np.asarray(b_proj, dtype=np.float32)
    y, _ = run(x, w_qkv, w_proj, b_proj, trace=False)
    return y
